# revision 1
# baseline (speedup 1.0000x reference)
"""Trainium2 Bass kernel for AtomEmbedding:
    h = LayerNorm(emb[z] + W2 @ silu(W1 @ x + b1) + b2) * gamma + beta

Strategy (pure data parallel over the packed atom axis):
  - N = 524288 atoms sharded 65536/core over 8 NeuronCores; all params replicated.
  - All matmuls run on the PE in bf16 (fp32 is 4 cyc/row on TRN2; bf16 is 1).
  - mm1 computes p^T = w1_aug^T @ x_aug^T in D-on-partitions layout so that
    silu(p)^T can be used directly as the stationary operand of mm2
    (no transposes anywhere). b1 is folded in via an ones-row in x_aug.
  - The embedding gather is a one-hot matmul accumulated into the same PSUM
    tile as mm2, so h = emb'[z] + p forms entirely inside PSUM (b2 is folded
    into emb' = emb + b2 host-side). The one-hot (types x atoms, exact in
    bf16) is precomputed on the host and streamed from DRAM (measured ~18%
    faster than building it on-device via gpsimd broadcast + compare).
  - LayerNorm per 128-atom tile: bn_stats/bn_aggr on the DVE over PSUM h,
    rstd = rsqrt(var+eps) computed on the DVE via the 0x5f3759df bit-trick
    seed + 2 Newton steps (keeps ACT on a single table set - an ACT Sqrt
    would thrash spline-table loads against Silu every group), then
    normalize as h*rstd + (-mu*rstd) split 3:1 between ACT and DVE.
    gamma/beta are applied only when not the trivial ones/zeros.
"""

import os
import sys

import numpy as np

for _p in ("/opt/trn_rl_repo", "/opt/pypackages"):
    if _p not in sys.path and os.path.isdir(_p):
        sys.path.append(_p)

N = 524288
D = 256
NT = 100  # number of atom types
NCORES = 8
NPC = N // NCORES  # atoms per core
A = int(os.environ.get("ATOMEMB_A", "512"))  # atoms per group
TPG = A // 128  # 128-atom tiles per group
EPS = 1e-5

# matmul operand dtype: "f32r" (fp32 storage, fast PE mode) or "bf16"
MM_MODE = os.environ.get("ATOMEMB_MM_MODE", "bf16")
# how many of each 4 normalize tiles run on ACT (rest on DVE)
NORM_ACT_TILES = int(os.environ.get("ATOMEMB_NORM_ACT", "3"))
PSA_BUFS = int(os.environ.get("ATOMEMB_PSA", "1"))
PSB_BUFS = int(os.environ.get("ATOMEMB_PSB", "6"))
BUFS_IN = int(os.environ.get("ATOMEMB_BIN", "4"))
BUFS_OH = int(os.environ.get("ATOMEMB_BOH", "3"))
BUFS_S = int(os.environ.get("ATOMEMB_BS", "6"))
BUFS_O = int(os.environ.get("ATOMEMB_BO", "4"))
# stream a host-precomputed one-hot from DRAM instead of building it
# on-device with gpsimd partition_broadcast + compare
HOST_OH = os.environ.get("ATOMEMB_HOST_OH", "1") == "1"
# issue the big output store via gpsimd SWDGE (frees the HWDGE rings)
SWDGE_OUT = os.environ.get("ATOMEMB_SWDGE_OUT", "0") == "1"

_MODULE_CACHE: dict = {}


def _build_module(npc: int, apply_affine: bool, mm_mode: str,
                  sim_safe_silu: bool = False):
    """Build + compile the Bass module for one core's slice (npc atoms).

    sim_safe_silu: CoreSim doesn't implement the Silu activation; when True,
    emit Sigmoid + multiply instead (slower, only used for simulation runs).
    """
    from contextlib import ExitStack

    import concourse.bacc as bacc
    import concourse.tile as tile
    from concourse import mybir

    f32 = mybir.dt.float32
    if mm_mode == "f32r":
        # float32r tiles end-to-end: every producer of a matmul operand
        # (DMA from an f32r DRAM tensor, ACT silu, DVE compare) then counts
        # as rounding to f32r for the BIR verifier.
        st_dt = mybir.dt.float32r
    elif mm_mode == "bf16":
        st_dt = mybir.dt.bfloat16
    else:
        raise ValueError(mm_mode)

    ngroups = npc // A

    nc = bacc.Bacc(
        "TRN2",
        target_bir_lowering=False,
        debug=False,
        enable_asserts=False,
        num_devices=NCORES,
    )

    # Per-core inputs (host pre-transposed / folded):
    #   xT:   [4, npc]  rows = (x0, x1, x2, 1)          -> moving operand of mm1
    #   zrow: [1, npc]  z as float
    #   w1c:  [4, D]    [w1; b1]                        -> stationary of mm1
    #   w2a:  [2, 128, D] w2 split into two k-chunks    -> moving of mm2
    #   emba: [NT, D]   emb + b2                        -> moving of gather-mm
    #   iotac:[NT, 1]   0..NT-1 column
    xT = nc.dram_tensor("xT", [4, npc], st_dt, kind="ExternalInput")
    if HOST_OH:
        ohT = nc.dram_tensor("ohT", [NT, npc], st_dt, kind="ExternalInput")
    else:
        zrow = nc.dram_tensor("zrow", [1, npc], st_dt, kind="ExternalInput")
    w1c = nc.dram_tensor("w1c", [4, D], st_dt, kind="ExternalInput")
    w2a = nc.dram_tensor("w2a", [128, 2, D], st_dt, kind="ExternalInput")
    emba = nc.dram_tensor("emba", [NT, D], st_dt, kind="ExternalInput")
    iotac = nc.dram_tensor("iotac", [NT, 1], f32, kind="ExternalInput")
    if apply_affine:
        gmb = nc.dram_tensor("gmb", [128, D], f32, kind="ExternalInput")
        btb = nc.dram_tensor("btb", [128, D], f32, kind="ExternalInput")
    out = nc.dram_tensor("out", [npc, D], f32, kind="ExternalOutput")

    def mm(ap):
        return ap

    with tile.TileContext(nc) as tc:
        with ExitStack() as ctx:
            consts = ctx.enter_context(tc.tile_pool(name="consts", bufs=1))
            xpool = ctx.enter_context(tc.tile_pool(name="xpool", bufs=BUFS_IN))
            zpool = ctx.enter_context(tc.tile_pool(name="zpool", bufs=BUFS_IN))
            zbpool = ctx.enter_context(tc.tile_pool(name="zbpool", bufs=BUFS_OH))
            ohpool = ctx.enter_context(tc.tile_pool(name="ohpool", bufs=BUFS_OH))
            spool = ctx.enter_context(tc.tile_pool(name="spool", bufs=BUFS_S))
            stpool = ctx.enter_context(tc.tile_pool(name="stpool", bufs=12))
            opool = ctx.enter_context(tc.tile_pool(name="opool", bufs=BUFS_O))
            psA = ctx.enter_context(tc.tile_pool(name="psA", bufs=PSA_BUFS, space="PSUM"))
            psB = ctx.enter_context(tc.tile_pool(name="psB", bufs=PSB_BUFS, space="PSUM"))

            # ---- load constants once ----
            sb_w1 = consts.tile([4, D], st_dt)
            nc.sync.dma_start(out=sb_w1[:], in_=w1c[:])
            sb_w2 = consts.tile([128, 2, D], st_dt)
            nc.sync.dma_start(out=sb_w2[:], in_=w2a[:])
            sb_emb = consts.tile([NT, D], st_dt)
            nc.sync.dma_start(out=sb_emb[:], in_=emba[:])
            sb_iota = consts.tile([NT, 1], f32)
            nc.sync.dma_start(out=sb_iota[:], in_=iotac[:])
            sb_eps = consts.tile([128, 1], f32)
            nc.vector.memset(sb_eps[:], EPS)
            sb_magic_i = consts.tile([128, TPG], mybir.dt.int32)
            nc.vector.memset(sb_magic_i[:], 0x5F3759DF)
            sb_one_i = consts.tile([128, TPG], mybir.dt.int32)
            nc.vector.memset(sb_one_i[:], 1)
            if apply_affine:
                sb_gmb = consts.tile([128, D], f32)
                nc.sync.dma_start(out=sb_gmb[:], in_=gmb[:])
                sb_btb = consts.tile([128, D], f32)
                nc.sync.dma_start(out=sb_btb[:], in_=btb[:])

            for g in range(ngroups):
                a0 = g * A
                # ---- loads (ACT HWDGE ring; stores own the SP ring) ----
                xtt = xpool.tile([4, A], st_dt, tag="xtt")
                nc.scalar.dma_start(out=xtt[:], in_=xT[:, a0 : a0 + A])
                xt = xtt[:]
                oh = ohpool.tile([NT, A], st_dt, tag="oh")
                if HOST_OH:
                    # one-hot^T streamed from DRAM (host-precomputed)
                    nc.scalar.dma_start(out=oh[:], in_=ohT[:, a0 : a0 + A])
                else:
                    zt = zpool.tile([1, A], st_dt, tag="zt")
                    nc.scalar.dma_start(out=zt[:], in_=zrow[:, a0 : a0 + A])
                    zb = zbpool.tile([NT, A], st_dt, tag="zb")
                    nc.gpsimd.partition_broadcast(zb[:], zt[:], channels=NT)
                    nc.gpsimd.tensor_scalar(
                        out=oh[:],
                        in0=zb[:],
                        scalar1=sb_iota[:],
                        scalar2=None,
                        op0=mybir.AluOpType.is_equal,
                    )

                # ---- mm1: p^T chunks [128, A] (D on partitions) ----
                pT0 = psA.tile([128, A], f32, tag="pT0")
                pT1 = psA.tile([128, A], f32, tag="pT1")
                nc.tensor.matmul(pT0[:], mm(sb_w1[:, 0:128]), mm(xt),
                                 start=True, stop=True)
                nc.tensor.matmul(pT1[:], mm(sb_w1[:, 128:256]), mm(xt),
                                 start=True, stop=True)

                # ---- silu on ACT: s^T = Silu(p^T) ----
                s0 = spool.tile([128, A], st_dt, tag="s0")
                s1 = spool.tile([128, A], st_dt, tag="s1")
                if sim_safe_silu:
                    for ps, s in ((pT0, s0), (pT1, s1)):
                        sg = spool.tile([128, A], f32, tag="sg")
                        nc.scalar.activation(sg[:], ps[:],
                                             mybir.ActivationFunctionType.Sigmoid)
                        nc.vector.tensor_mul(s[:], sg[:], ps[:])
                else:
                    nc.scalar.activation(s0[:], pT0[:],
                                         mybir.ActivationFunctionType.Silu)
                    nc.scalar.activation(s1[:], pT1[:],
                                         mybir.ActivationFunctionType.Silu)

                # ---- mm2 + embedding gather accumulate: h tiles in PSUM ----
                hs = []
                for t in range(TPG):
                    c = t * 128
                    h = psB.tile([128, D], f32, tag="h")
                    hs.append(h)
                    nc.tensor.matmul(h[:], mm(s0[:, c : c + 128]),
                                     mm(sb_w2[:, 0, :]), start=True, stop=False)
                    nc.tensor.matmul(h[:], mm(s1[:, c : c + 128]),
                                     mm(sb_w2[:, 1, :]), start=False, stop=False)
                    nc.tensor.matmul(h[:], mm(oh[:, c : c + 128]),
                                     mm(sb_emb[:]), start=False, stop=True)

                # ---- LayerNorm stats (DVE), batched scalar tail ----
                stg = stpool.tile([128, TPG, 6], f32, tag="stg")
                for t in range(TPG):
                    nc.vector.bn_stats(out=stg[:, t, :], in_=hs[t][:])
                mvg = stpool.tile([128, TPG, 2], f32, tag="mvg")
                for t in range(TPG):
                    nc.vector.bn_aggr(out=mvg[:, t, :], in_=stg[:, t, :])
                # rs = rsqrt(var + eps) via bit-trick seed + 2 Newton steps
                # (keeps ACT on a single table set: no Sqrt -> no reloads)
                i32 = mybir.dt.int32
                w = stpool.tile([128, TPG], f32, tag="w")
                nc.vector.tensor_scalar(
                    out=w[:], in0=mvg[:, :, 1], scalar1=EPS, scalar2=None,
                    op0=mybir.AluOpType.add)
                y = stpool.tile([128, TPG], f32, tag="y")
                yi = y[:].bitcast(i32)
                t1 = stpool.tile([128, TPG], f32, tag="t1")
                # yi = magic - (w_i32 >> 1)
                nc.vector.tensor_tensor(
                    out=t1[:].bitcast(i32), in0=w[:].bitcast(i32),
                    in1=sb_one_i[:, 0:TPG],
                    op=mybir.AluOpType.logical_shift_right)
                nc.vector.tensor_tensor(
                    out=yi, in0=sb_magic_i[:, 0:TPG],
                    in1=t1[:].bitcast(i32),
                    op=mybir.AluOpType.subtract)
                for _ in range(2):
                    nc.vector.tensor_mul(t1[:], y[:], y[:])
                    nc.vector.scalar_tensor_tensor(
                        out=t1[:], in0=t1[:], scalar=-0.5, in1=w[:],
                        op0=mybir.AluOpType.mult, op1=mybir.AluOpType.mult)
                    nc.vector.scalar_tensor_tensor(
                        out=y[:], in0=t1[:], scalar=1.5, in1=y[:],
                        op0=mybir.AluOpType.add, op1=mybir.AluOpType.mult)
                rs = y
                cc = stpool.tile([128, TPG], f32, tag="cc")
                nc.vector.scalar_tensor_tensor(
                    out=cc[:], in0=mvg[:, :, 0], scalar=-1.0, in1=rs[:],
                    op0=mybir.AluOpType.mult, op1=mybir.AluOpType.mult,
                )

                # ---- normalize; 3 tiles on ACT, 1 on DVE (engine balance) ----
                og = opool.tile([128, TPG, D], f32, tag="og")
                for t in range(TPG):
                    h = hs[t][:]
                    o = og[:, t, :]
                    if t % 4 < NORM_ACT_TILES:
                        # ACT: o = h*rs + (-mu*rs)
                        nc.scalar.activation(
                            o, h,
                            mybir.ActivationFunctionType.Identity,
                            bias=cc[:, t : t + 1], scale=rs[:, t : t + 1])
                    else:
                        # DVE: o = h*rs + cc
                        nc.vector.tensor_scalar(
                            out=o, in0=h,
                            scalar1=rs[:, t : t + 1], scalar2=cc[:, t : t + 1],
                            op0=mybir.AluOpType.mult,
                            op1=mybir.AluOpType.add,
                        )
                    if apply_affine:
                        nc.vector.tensor_mul(o, o, sb_gmb[:])
                        nc.vector.tensor_add(o, o, sb_btb[:])

                # one batched 512KB store: DRAM row a0 + t*128 + p <- og[p, t, :]
                out_view = out[a0 : a0 + A, :].rearrange(
                    "(t p) d -> p t d", p=128)
                store_eng = nc.gpsimd if SWDGE_OUT else nc.sync
                store_eng.dma_start(out=out_view, in_=og[:])

    nc.compile()
    return nc


def _get_module(npc: int, apply_affine: bool, mm_mode: str = MM_MODE,
                sim_safe_silu: bool = False):
    key = (npc, apply_affine, mm_mode, sim_safe_silu)
    if key not in _MODULE_CACHE:
        _MODULE_CACHE[key] = _build_module(npc, apply_affine, mm_mode,
                                           sim_safe_silu)
    return _MODULE_CACHE[key]


def _prep_inputs(z, x, emb, w1, b1, w2, b2, gamma, beta, npc, apply_affine,
                 mm_mode: str = MM_MODE):
    """Host-side folding/transposes; returns per-core in_maps."""
    if mm_mode == "f32r":
        st = np.float32
    else:
        import ml_dtypes

        st = ml_dtypes.bfloat16

    z = np.asarray(z)
    x = np.asarray(x, dtype=np.float32)
    n = z.shape[0]

    xT = np.empty((4, n), dtype=np.float32)
    xT[0:3] = x.T
    xT[3] = 1.0
    xT = xT.astype(st)
    zi = np.asarray(z).astype(np.int64)
    if HOST_OH:
        ohT = (zi[None, :] == np.arange(NT, dtype=np.int64)[:, None]).astype(st)
    else:
        zrow = zi.astype(np.float32).reshape(1, n).astype(st)
    w1a = np.concatenate([np.asarray(w1, np.float32),
                          np.asarray(b1, np.float32).reshape(1, D)], axis=0)
    w1c = w1a.astype(st)
    w2f = np.asarray(w2, np.float32)
    w2a = np.stack([w2f[0:128], w2f[128:256]], axis=1).astype(st)
    emba = (np.asarray(emb, np.float32)
            + np.asarray(b2, np.float32).reshape(1, D)).astype(st)
    iotac = np.arange(NT, dtype=np.float32).reshape(NT, 1)

    common = {"w1c": w1c, "w2a": w2a, "emba": emba, "iotac": iotac}
    if apply_affine:
        common["gmb"] = np.broadcast_to(
            np.asarray(gamma, np.float32).reshape(1, D), (128, D)).copy()
        common["btb"] = np.broadcast_to(
            np.asarray(beta, np.float32).reshape(1, D), (128, D)).copy()

    in_maps = []
    for c in range(NCORES):
        s = slice(c * npc, (c + 1) * npc)
        m = {"xT": np.ascontiguousarray(xT[:, s]), **common}
        if HOST_OH:
            m["ohT"] = np.ascontiguousarray(ohT[:, s])
        else:
            m["zrow"] = np.ascontiguousarray(zrow[:, s])
        in_maps.append(m)
    return in_maps


def _run(in_maps, nc, trace=False):
    from concourse.bass_interp import get_hw_module
    from concourse.bass_utils import run_bass_kernel_spmd

    old_m = nc.m
    nc.m = get_hw_module(nc.m)
    try:
        res = run_bass_kernel_spmd(
            nc, in_maps, core_ids=list(range(NCORES)), trace=trace
        )
    finally:
        nc.m = old_m
    return res


def kernel(z, x, emb, w1, b1, w2, b2, gamma, beta):
    z = np.asarray(z)
    x = np.asarray(x)
    assert z.shape[0] == N and x.shape == (N, 3), (z.shape, x.shape)

    apply_affine = not (
        np.all(np.asarray(gamma) == 1.0) and np.all(np.asarray(beta) == 0.0)
    )
    nc = _get_module(NPC, apply_affine)
    in_maps = _prep_inputs(z, x, emb, w1, b1, w2, b2, gamma, beta,
                           NPC, apply_affine)
    res = _run(in_maps, nc, trace=False)
    out = np.concatenate([r["out"] for r in res.results], axis=0)
    return out.astype(np.float32)



# revision 7
# speedup vs baseline: 1.0079x; 1.0079x over previous
"""Trainium2 Bass kernel for AtomEmbedding:
    h = LayerNorm(emb[z] + W2 @ silu(W1 @ x + b1) + b2) * gamma + beta

v2 design (engine-balanced, bf16 output):
  - N = 524288 atoms sharded 65536/core over 8 NeuronCores; params replicated.
  - Groups of A=512 atoms; 2 groups in flight (PSUM: 2x pT(2 banks) +
    2x hg(2 banks) = 8 banks exactly).
  - mm1 computes p^T = [w1;b1]^T @ [x;1]^T into ONE PSUM tile [128,2,512];
    silu is a single ACT op over FD=1024 -> s bf16 SBUF.
  - mm2 (2 k-chunks) + one-hot embedding gather accumulate h tiles
    [128 atoms, 256] packed into one PSUM tile [128,4,256] (b2 folded into
    emb host-side; one-hot^T streamed bf16 from DRAM).
  - LayerNorm stats: 2x batched bn_stats ([128,2,256] -> [128,2,6], the
    512-elem HW cap) + 4x bn_aggr -> mean/var. rsqrt(var+eps) via
    0x5f3759df-seed Newton + cc = -mu*rs on the (otherwise idle) GpSimd
    engine, freeing the DVE.
  - Normalize o = h*rs + cc: tiles split between ACT (Identity, scale/bias
    per-partition APs) and DVE (tensor_scalar), alternating 3/1 and 2/2 per
    group to balance ACT (silu-loaded) vs DVE (stats-loaded).
  - Output written bf16 (halves store traffic; rel-err budget allows);
    host casts back to f32.
"""

import os
import sys

import numpy as np

for _p in ("/opt/trn_rl_repo", "/opt/pypackages"):
    if _p not in sys.path and os.path.isdir(_p):
        sys.path.append(_p)

N = 524288
D = 256
NT = 100  # number of atom types
NCORES = 8
NPC = N // NCORES  # atoms per core
A = int(os.environ.get("ATOMEMB_A", "512"))  # atoms per group
TPG = A // 128  # 128-atom tiles per group
EPS = 1e-5

# knobs (defaults = the design; env lets HW A/B without editing)
OUT_BF16 = os.environ.get("ATOMEMB_OUT_BF16", "1") == "1"
EPI_ENGINE = os.environ.get("ATOMEMB_EPI", "gpsimd")  # gpsimd | dve
NEWTON_ITERS = int(os.environ.get("ATOMEMB_NEWTON", "2"))
# normalize tiles on ACT per group, cycled (rest on DVE)
NORM_ACT_CYCLE = tuple(
    int(x) for x in os.environ.get("ATOMEMB_NORM_ACT", "3").split(","))
BUFS_IN = int(os.environ.get("ATOMEMB_BIN", "3"))
BUFS_OH = int(os.environ.get("ATOMEMB_BOH", "3"))
BUFS_S = int(os.environ.get("ATOMEMB_BS", "2"))
BUFS_O = int(os.environ.get("ATOMEMB_BO", "3"))
PSA_BUFS = int(os.environ.get("ATOMEMB_PSA", "2"))
PSB_BUFS = int(os.environ.get("ATOMEMB_PSB", "2"))

_MODULE_CACHE: dict = {}


def _build_module(npc: int, apply_affine: bool, sim_safe_silu: bool = False):
    """Build + compile the Bass module for one core's slice (npc atoms).

    sim_safe_silu: CoreSim doesn't implement the Silu activation; when True,
    emit Sigmoid + multiply instead (slower, only used for simulation runs).
    """
    from contextlib import ExitStack

    import concourse.bacc as bacc
    import concourse.tile as tile
    from concourse import mybir

    f32 = mybir.dt.float32
    bf16 = mybir.dt.bfloat16
    i32 = mybir.dt.int32
    out_dt = bf16 if OUT_BF16 else f32

    ngroups = npc // A
    assert npc % A == 0

    nc = bacc.Bacc(
        "TRN2",
        target_bir_lowering=False,
        debug=False,
        enable_asserts=False,
        num_devices=NCORES,
    )

    # Per-core inputs (host pre-transposed / folded):
    #   xT:   [4, npc]  rows = (x0, x1, x2, 1)     -> moving operand of mm1
    #   ohT:  [NT, npc] one-hot^T                   -> stationary of gather-mm
    #   w1c:  [4, D]    [w1; b1]                    -> stationary of mm1
    #   w2a:  [128, 2, D] w2 split into two k-chunks -> moving of mm2
    #   emba: [NT, D]   emb + b2                    -> moving of gather-mm
    xT = nc.dram_tensor("xT", [4, npc], bf16, kind="ExternalInput")
    ohT = nc.dram_tensor("ohT", [NT, npc], bf16, kind="ExternalInput")
    w1c = nc.dram_tensor("w1c", [4, D], bf16, kind="ExternalInput")
    w2a = nc.dram_tensor("w2a", [128, 2, D], bf16, kind="ExternalInput")
    emba = nc.dram_tensor("emba", [NT, D], bf16, kind="ExternalInput")
    if apply_affine:
        gmb = nc.dram_tensor("gmb", [128, D], f32, kind="ExternalInput")
        btb = nc.dram_tensor("btb", [128, D], f32, kind="ExternalInput")
    out = nc.dram_tensor("out", [npc, D], out_dt, kind="ExternalOutput")

    with tile.TileContext(nc) as tc:
        with ExitStack() as ctx:
            consts = ctx.enter_context(tc.tile_pool(name="consts", bufs=1))
            xpool = ctx.enter_context(tc.tile_pool(name="xpool", bufs=BUFS_IN))
            ohpool = ctx.enter_context(tc.tile_pool(name="ohpool", bufs=BUFS_OH))
            spool = ctx.enter_context(tc.tile_pool(name="spool", bufs=BUFS_S))
            stpool = ctx.enter_context(tc.tile_pool(name="stpool", bufs=4))
            opool = ctx.enter_context(tc.tile_pool(name="opool", bufs=BUFS_O))
            psA = ctx.enter_context(
                tc.tile_pool(name="psA", bufs=PSA_BUFS, space="PSUM"))
            psB = ctx.enter_context(
                tc.tile_pool(name="psB", bufs=PSB_BUFS, space="PSUM"))

            # ---- load constants once ----
            sb_w1 = consts.tile([4, D], bf16)
            nc.sync.dma_start(out=sb_w1[:], in_=w1c[:])
            sb_w2 = consts.tile([128, 2, D], bf16)
            nc.sync.dma_start(out=sb_w2[:], in_=w2a[:])
            sb_emb = consts.tile([NT, D], bf16)
            nc.sync.dma_start(out=sb_emb[:], in_=emba[:])
            sb_magic_i = consts.tile([128, TPG], i32)
            nc.vector.memset(sb_magic_i[:], 0x5F3759DF)
            sb_one_i = consts.tile([128, TPG], i32)
            nc.vector.memset(sb_one_i[:], 1)
            sb_eps = consts.tile([128, TPG], f32)
            nc.vector.memset(sb_eps[:], EPS)
            sb_nhalf = consts.tile([128, TPG], f32)
            nc.vector.memset(sb_nhalf[:], -0.5)
            sb_1p5 = consts.tile([128, TPG], f32)
            nc.vector.memset(sb_1p5[:], 1.5)
            sb_neg1 = consts.tile([128, TPG], f32)
            nc.vector.memset(sb_neg1[:], -1.0)
            if apply_affine:
                sb_gmb = consts.tile([128, D], f32)
                nc.sync.dma_start(out=sb_gmb[:], in_=gmb[:])
                sb_btb = consts.tile([128, D], f32)
                nc.sync.dma_start(out=sb_btb[:], in_=btb[:])

            # epilogue engine: ops on tiny [128, TPG] tiles
            ep = nc.gpsimd if EPI_ENGINE == "gpsimd" else nc.vector

            for g in range(ngroups):
                a0 = g * A
                # ---- loads (ACT HWDGE ring; stores own the SP ring) ----
                xt = xpool.tile([4, A], bf16, tag="xt")
                nc.scalar.dma_start(out=xt[:], in_=xT[:, a0 : a0 + A])
                oh = ohpool.tile([NT, A], bf16, tag="oh")
                nc.scalar.dma_start(out=oh[:], in_=ohT[:, a0 : a0 + A])

                # ---- mm1: p^T [128, 2, A] (D on partitions, one tile) ----
                pT = psA.tile([128, 2, A], f32, tag="pT")
                nc.tensor.matmul(pT[:, 0, :], sb_w1[:, 0:128], xt[:],
                                 start=True, stop=True)
                nc.tensor.matmul(pT[:, 1, :], sb_w1[:, 128:256], xt[:],
                                 start=True, stop=True)

                # ---- silu on ACT (single op over FD=2A) ----
                s = spool.tile([128, 2, A], bf16, tag="s")
                if sim_safe_silu:
                    sg = spool.tile([128, 2, A], f32, tag="sg")
                    nc.scalar.activation(sg[:], pT[:],
                                         mybir.ActivationFunctionType.Sigmoid)
                    nc.vector.tensor_mul(s[:], sg[:], pT[:])
                else:
                    nc.scalar.activation(s[:], pT[:],
                                         mybir.ActivationFunctionType.Silu)

                # ---- mm2 + embedding gather: h tiles in one PSUM tile ----
                hg = psB.tile([128, TPG, D], f32, tag="hg")
                for t in range(TPG):
                    c = t * 128
                    nc.tensor.matmul(hg[:, t, :], s[:, 0, c : c + 128],
                                     sb_w2[:, 0, :], start=True, stop=False)
                    nc.tensor.matmul(hg[:, t, :], s[:, 1, c : c + 128],
                                     sb_w2[:, 1, :], start=False, stop=False)
                    nc.tensor.matmul(hg[:, t, :], oh[:, c : c + 128],
                                     sb_emb[:], start=False, stop=True)

                # ---- LayerNorm stats on DVE (bn_stats is single-group) ----
                stg = stpool.tile([128, TPG, 6], f32, tag="stg")
                for t in range(TPG):
                    nc.vector.bn_stats(out=stg[:, t, :], in_=hg[:, t, :])
                mvg = stpool.tile([128, TPG, 2], f32, tag="mvg")
                for t in range(TPG):
                    nc.vector.bn_aggr(out=mvg[:, t, :], in_=stg[:, t, :])

                # ---- epilogue: rs = rsqrt(var+eps), cc = -mu*rs ----
                # (bit-trick seed + Newton; mostly on GpSimd to free the DVE.
                #  Pool only supports plain tensor_tensor -> const-tile form;
                #  the int32 shift seed is DVE-only on TRN2.)
                TT = mybir.AluOpType
                w = stpool.tile([128, TPG], f32, tag="w")
                ep.tensor_tensor(out=w[:], in0=mvg[:, :, 1],
                                 in1=sb_eps[:], op=TT.add)
                y = stpool.tile([128, TPG], f32, tag="y")
                t1 = stpool.tile([128, TPG], f32, tag="t1")
                # yi = magic - (w_i32 >> 1)
                nc.vector.tensor_tensor(
                    out=t1[:].bitcast(i32), in0=w[:].bitcast(i32),
                    in1=sb_one_i[:, 0:TPG],
                    op=TT.logical_shift_right)
                nc.vector.tensor_tensor(
                    out=y[:].bitcast(i32), in0=sb_magic_i[:, 0:TPG],
                    in1=t1[:].bitcast(i32),
                    op=TT.subtract)
                # Newton: y <- y * (1.5 - 0.5*w*y*y)
                for _ in range(NEWTON_ITERS):
                    ep.tensor_tensor(out=t1[:], in0=y[:], in1=y[:], op=TT.mult)
                    ep.tensor_tensor(out=t1[:], in0=t1[:], in1=w[:], op=TT.mult)
                    ep.tensor_tensor(out=t1[:], in0=t1[:], in1=sb_nhalf[:],
                                     op=TT.mult)
                    ep.tensor_tensor(out=t1[:], in0=t1[:], in1=sb_1p5[:],
                                     op=TT.add)
                    ep.tensor_tensor(out=y[:], in0=t1[:], in1=y[:], op=TT.mult)
                rs = y
                cc = stpool.tile([128, TPG], f32, tag="cc")
                ep.tensor_tensor(out=cc[:], in0=mvg[:, :, 0], in1=rs[:],
                                 op=TT.mult)
                ep.tensor_tensor(out=cc[:], in0=cc[:], in1=sb_neg1[:],
                                 op=TT.mult)

                # ---- normalize: o = h*rs + cc, split ACT/DVE ----
                n_act = NORM_ACT_CYCLE[g % len(NORM_ACT_CYCLE)]
                og = opool.tile([128, TPG, D], out_dt, tag="og")
                for t in range(TPG):
                    h = hg[:, t, :]
                    o = og[:, t, :]
                    if t < n_act:
                        nc.scalar.activation(
                            o, h,
                            mybir.ActivationFunctionType.Identity,
                            bias=cc[:, t : t + 1], scale=rs[:, t : t + 1])
                    else:
                        nc.vector.tensor_scalar(
                            out=o, in0=h,
                            scalar1=rs[:, t : t + 1], scalar2=cc[:, t : t + 1],
                            op0=mybir.AluOpType.mult,
                            op1=mybir.AluOpType.add,
                        )
                    if apply_affine:
                        nc.vector.tensor_mul(o, o, sb_gmb[:])
                        nc.vector.tensor_add(o, o, sb_btb[:])

                # one batched store: DRAM row a0 + t*128 + p <- og[p, t, :]
                out_view = out[a0 : a0 + A, :].rearrange(
                    "(t p) d -> p t d", p=128)
                nc.sync.dma_start(out=out_view, in_=og[:])

    nc.compile()
    return nc


def _get_module(npc: int, apply_affine: bool, sim_safe_silu: bool = False):
    key = (npc, apply_affine, sim_safe_silu)
    if key not in _MODULE_CACHE:
        _MODULE_CACHE[key] = _build_module(npc, apply_affine, sim_safe_silu)
    return _MODULE_CACHE[key]


def _prep_inputs(z, x, emb, w1, b1, w2, b2, gamma, beta, npc, apply_affine):
    """Host-side folding/transposes; returns per-core in_maps."""
    import ml_dtypes

    st = ml_dtypes.bfloat16

    z = np.asarray(z)
    x = np.asarray(x, dtype=np.float32)
    n = z.shape[0]

    xT = np.empty((4, n), dtype=np.float32)
    xT[0:3] = x.T
    xT[3] = 1.0
    xT = xT.astype(st)
    zi = np.asarray(z).astype(np.int64)
    ohT = (zi[None, :] == np.arange(NT, dtype=np.int64)[:, None]).astype(st)
    w1a = np.concatenate([np.asarray(w1, np.float32),
                          np.asarray(b1, np.float32).reshape(1, D)], axis=0)
    w1c = w1a.astype(st)
    w2f = np.asarray(w2, np.float32)
    w2a = np.stack([w2f[0:128], w2f[128:256]], axis=1).astype(st)
    emba = (np.asarray(emb, np.float32)
            + np.asarray(b2, np.float32).reshape(1, D)).astype(st)

    common = {"w1c": w1c, "w2a": w2a, "emba": emba}
    if apply_affine:
        common["gmb"] = np.broadcast_to(
            np.asarray(gamma, np.float32).reshape(1, D), (128, D)).copy()
        common["btb"] = np.broadcast_to(
            np.asarray(beta, np.float32).reshape(1, D), (128, D)).copy()

    in_maps = []
    for c in range(NCORES):
        s = slice(c * npc, (c + 1) * npc)
        m = {"xT": np.ascontiguousarray(xT[:, s]),
             "ohT": np.ascontiguousarray(ohT[:, s]), **common}
        in_maps.append(m)
    return in_maps


def _run(in_maps, nc, trace=False):
    from concourse.bass_interp import get_hw_module
    from concourse.bass_utils import run_bass_kernel_spmd

    old_m = nc.m
    nc.m = get_hw_module(nc.m)
    try:
        res = run_bass_kernel_spmd(
            nc, in_maps, core_ids=list(range(NCORES)), trace=trace
        )
    finally:
        nc.m = old_m
    return res


def kernel(z, x, emb, w1, b1, w2, b2, gamma, beta):
    z = np.asarray(z)
    x = np.asarray(x)
    assert z.shape[0] == N and x.shape == (N, 3), (z.shape, x.shape)

    apply_affine = not (
        np.all(np.asarray(gamma) == 1.0) and np.all(np.asarray(beta) == 0.0)
    )
    nc = _get_module(NPC, apply_affine)
    in_maps = _prep_inputs(z, x, emb, w1, b1, w2, b2, gamma, beta,
                           NPC, apply_affine)
    res = _run(in_maps, nc, trace=False)
    out = np.concatenate([np.asarray(r["out"]) for r in res.results], axis=0)
    return out.astype(np.float32)


# revision 10
# speedup vs baseline: 2.4953x; 2.4758x over previous
"""Trainium2 Bass kernel for AtomEmbedding:
    h = LayerNorm(emb[z] + W2 @ silu(W1 @ x + b1) + b2) * gamma + beta

v2 design (engine-balanced, bf16 output):
  - N = 524288 atoms sharded 65536/core over 8 NeuronCores; params replicated.
  - Groups of A=512 atoms; 2 groups in flight (PSUM: 2x pT(2 banks) +
    2x hg(2 banks) = 8 banks exactly).
  - mm1 computes p^T = [w1;b1]^T @ [x;1]^T into ONE PSUM tile [128,2,512];
    silu is a single ACT op over FD=1024 -> s bf16 SBUF.
  - mm2 (2 k-chunks) + one-hot embedding gather accumulate h tiles
    [128 atoms, 256] packed into one PSUM tile [128,4,256] (b2 folded into
    emb host-side; one-hot^T streamed bf16 from DRAM).
  - LayerNorm stats: 2x batched bn_stats ([128,2,256] -> [128,2,6], the
    512-elem HW cap) + 4x bn_aggr -> mean/var. rsqrt(var+eps) via
    0x5f3759df-seed Newton + cc = -mu*rs on the (otherwise idle) GpSimd
    engine, freeing the DVE.
  - Normalize o = h*rs + cc: tiles split between ACT (Identity, scale/bias
    per-partition APs) and DVE (tensor_scalar), alternating 3/1 and 2/2 per
    group to balance ACT (silu-loaded) vs DVE (stats-loaded).
  - Output written bf16 (halves store traffic; rel-err budget allows);
    host casts back to f32.
"""

import os
import sys

import numpy as np

for _p in ("/opt/trn_rl_repo", "/opt/pypackages"):
    if _p not in sys.path and os.path.isdir(_p):
        sys.path.append(_p)

N = 524288
D = 256
NT = 100  # number of atom types
NCORES = 8
NPC = N // NCORES  # atoms per core
A = int(os.environ.get("ATOMEMB_A", "512"))  # atoms per group
TPG = A // 128  # 128-atom tiles per group
EPS = 1e-5

# knobs (defaults = the design; env lets HW A/B without editing)
OUT_BF16 = os.environ.get("ATOMEMB_OUT_BF16", "1") == "1"
EPI_ENGINE = os.environ.get("ATOMEMB_EPI", "gpsimd")  # gpsimd | dve
NEWTON_ITERS = int(os.environ.get("ATOMEMB_NEWTON", "1"))
# normalize tiles on ACT per group, cycled (rest on DVE)
NORM_ACT_CYCLE = tuple(
    int(x) for x in os.environ.get("ATOMEMB_NORM_ACT", "3").split(","))
BUFS_IN = int(os.environ.get("ATOMEMB_BIN", "4"))
BUFS_OH = int(os.environ.get("ATOMEMB_BOH", "4"))
BUFS_S = int(os.environ.get("ATOMEMB_BS", "3"))
BUFS_O = int(os.environ.get("ATOMEMB_BO", "4"))
PSA_BUFS = int(os.environ.get("ATOMEMB_PSA", "1"))
PSB_BUFS = int(os.environ.get("ATOMEMB_PSB", "3"))
# engine issuing the input loads: sync (SP ring) frees the ACT sequencer
LOAD_ENG = os.environ.get("ATOMEMB_LOAD_ENG", "sync")

_MODULE_CACHE: dict = {}


def _build_module(npc: int, apply_affine: bool, sim_safe_silu: bool = False):
    """Build + compile the Bass module for one core's slice (npc atoms).

    sim_safe_silu: CoreSim doesn't implement the Silu activation; when True,
    emit Sigmoid + multiply instead (slower, only used for simulation runs).
    """
    from contextlib import ExitStack

    import concourse.bacc as bacc
    import concourse.tile as tile
    from concourse import mybir

    f32 = mybir.dt.float32
    bf16 = mybir.dt.bfloat16
    i32 = mybir.dt.int32
    out_dt = bf16 if OUT_BF16 else f32

    ngroups = npc // A
    assert npc % A == 0

    nc = bacc.Bacc(
        "TRN2",
        target_bir_lowering=False,
        debug=False,
        enable_asserts=False,
        num_devices=NCORES,
    )

    # Per-core inputs (host pre-transposed / folded):
    #   xT:   [4, npc]  rows = (x0, x1, x2, 1)     -> moving operand of mm1
    #   ohT:  [NT, npc] one-hot^T                   -> stationary of gather-mm
    #   w1c:  [4, D]    [w1; b1]                    -> stationary of mm1
    #   w2a:  [128, 2, D] w2 split into two k-chunks -> moving of mm2
    #   emba: [NT, D]   emb + b2                    -> moving of gather-mm
    xT = nc.dram_tensor("xT", [4, npc], bf16, kind="ExternalInput")
    ohT = nc.dram_tensor("ohT", [NT, npc], bf16, kind="ExternalInput")
    w1c = nc.dram_tensor("w1c", [4, D], bf16, kind="ExternalInput")
    w2a = nc.dram_tensor("w2a", [128, 2, D], bf16, kind="ExternalInput")
    emba = nc.dram_tensor("emba", [NT, D], bf16, kind="ExternalInput")
    if apply_affine:
        gmb = nc.dram_tensor("gmb", [128, D], f32, kind="ExternalInput")
        btb = nc.dram_tensor("btb", [128, D], f32, kind="ExternalInput")
    out = nc.dram_tensor("out", [npc, D], out_dt, kind="ExternalOutput")

    with tile.TileContext(nc) as tc:
        with ExitStack() as ctx:
            consts = ctx.enter_context(tc.tile_pool(name="consts", bufs=1))
            xpool = ctx.enter_context(tc.tile_pool(name="xpool", bufs=BUFS_IN))
            ohpool = ctx.enter_context(tc.tile_pool(name="ohpool", bufs=BUFS_OH))
            spool = ctx.enter_context(tc.tile_pool(name="spool", bufs=BUFS_S))
            stpool = ctx.enter_context(tc.tile_pool(name="stpool", bufs=4))
            opool = ctx.enter_context(tc.tile_pool(name="opool", bufs=BUFS_O))
            psA = ctx.enter_context(
                tc.tile_pool(name="psA", bufs=PSA_BUFS, space="PSUM"))
            psB = ctx.enter_context(
                tc.tile_pool(name="psB", bufs=PSB_BUFS, space="PSUM"))

            # ---- load constants once ----
            sb_w1 = consts.tile([4, D], bf16)
            nc.sync.dma_start(out=sb_w1[:], in_=w1c[:])
            sb_w2 = consts.tile([128, 2, D], bf16)
            nc.sync.dma_start(out=sb_w2[:], in_=w2a[:])
            sb_emb = consts.tile([NT, D], bf16)
            nc.sync.dma_start(out=sb_emb[:], in_=emba[:])
            sb_magic_i = consts.tile([128, TPG], i32)
            nc.vector.memset(sb_magic_i[:], 0x5F3759DF)
            sb_one_i = consts.tile([128, TPG], i32)
            nc.vector.memset(sb_one_i[:], 1)
            sb_eps = consts.tile([128, TPG], f32)
            nc.vector.memset(sb_eps[:], EPS)
            sb_nhalf = consts.tile([128, TPG], f32)
            nc.vector.memset(sb_nhalf[:], -0.5)
            sb_1p5 = consts.tile([128, TPG], f32)
            nc.vector.memset(sb_1p5[:], 1.5)
            sb_neg1 = consts.tile([128, TPG], f32)
            nc.vector.memset(sb_neg1[:], -1.0)
            if apply_affine:
                sb_gmb = consts.tile([128, D], f32)
                nc.sync.dma_start(out=sb_gmb[:], in_=gmb[:])
                sb_btb = consts.tile([128, D], f32)
                nc.sync.dma_start(out=sb_btb[:], in_=btb[:])

            # epilogue engine: ops on tiny [128, TPG] tiles
            ep = nc.gpsimd if EPI_ENGINE == "gpsimd" else nc.vector

            for g in range(ngroups):
                a0 = g * A
                load_eng = nc.sync if LOAD_ENG == "sync" else nc.scalar
                xt = xpool.tile([4, A], bf16, tag="xt")
                load_eng.dma_start(out=xt[:], in_=xT[:, a0 : a0 + A])
                oh = ohpool.tile([NT, A], bf16, tag="oh")
                load_eng.dma_start(out=oh[:], in_=ohT[:, a0 : a0 + A])

                # ---- mm1: p^T [128, 2, A] (D on partitions, one tile) ----
                pT = psA.tile([128, 2, A], f32, tag="pT")
                nc.tensor.matmul(pT[:, 0, :], sb_w1[:, 0:128], xt[:],
                                 start=True, stop=True)
                nc.tensor.matmul(pT[:, 1, :], sb_w1[:, 128:256], xt[:],
                                 start=True, stop=True)

                # ---- silu on ACT (single op over FD=2A) ----
                s = spool.tile([128, 2, A], bf16, tag="s")
                if sim_safe_silu:
                    sg = spool.tile([128, 2, A], f32, tag="sg")
                    nc.scalar.activation(sg[:], pT[:],
                                         mybir.ActivationFunctionType.Sigmoid)
                    nc.vector.tensor_mul(s[:], sg[:], pT[:])
                else:
                    nc.scalar.activation(s[:], pT[:],
                                         mybir.ActivationFunctionType.Silu)

                # ---- mm2 + embedding gather: h tiles in one PSUM tile ----
                hg = psB.tile([128, TPG, D], f32, tag="hg")
                for t in range(TPG):
                    c = t * 128
                    nc.tensor.matmul(hg[:, t, :], s[:, 0, c : c + 128],
                                     sb_w2[:, 0, :], start=True, stop=False)
                    nc.tensor.matmul(hg[:, t, :], s[:, 1, c : c + 128],
                                     sb_w2[:, 1, :], start=False, stop=False)
                    nc.tensor.matmul(hg[:, t, :], oh[:, c : c + 128],
                                     sb_emb[:], start=False, stop=True)

                # ---- LayerNorm stats on DVE (bn_stats is single-group) ----
                stg = stpool.tile([128, TPG, 6], f32, tag="stg")
                for t in range(TPG):
                    nc.vector.bn_stats(out=stg[:, t, :], in_=hg[:, t, :])
                mvg = stpool.tile([128, TPG, 2], f32, tag="mvg")
                for t in range(TPG):
                    nc.vector.bn_aggr(out=mvg[:, t, :], in_=stg[:, t, :])

                # ---- epilogue: rs = rsqrt(var+eps), cc = -mu*rs ----
                # (bit-trick seed + Newton; mostly on GpSimd to free the DVE.
                #  Pool only supports plain tensor_tensor -> const-tile form;
                #  the int32 shift seed is DVE-only on TRN2.)
                TT = mybir.AluOpType
                w = stpool.tile([128, TPG], f32, tag="w")
                ep.tensor_tensor(out=w[:], in0=mvg[:, :, 1],
                                 in1=sb_eps[:], op=TT.add)
                y = stpool.tile([128, TPG], f32, tag="y")
                t1 = stpool.tile([128, TPG], f32, tag="t1")
                # yi = magic - (w_i32 >> 1)
                nc.vector.tensor_tensor(
                    out=t1[:].bitcast(i32), in0=w[:].bitcast(i32),
                    in1=sb_one_i[:, 0:TPG],
                    op=TT.logical_shift_right)
                nc.vector.tensor_tensor(
                    out=y[:].bitcast(i32), in0=sb_magic_i[:, 0:TPG],
                    in1=t1[:].bitcast(i32),
                    op=TT.subtract)
                # Newton: y <- y * (1.5 - 0.5*w*y*y)
                for _ in range(NEWTON_ITERS):
                    ep.tensor_tensor(out=t1[:], in0=y[:], in1=y[:], op=TT.mult)
                    ep.tensor_tensor(out=t1[:], in0=t1[:], in1=w[:], op=TT.mult)
                    ep.tensor_tensor(out=t1[:], in0=t1[:], in1=sb_nhalf[:],
                                     op=TT.mult)
                    ep.tensor_tensor(out=t1[:], in0=t1[:], in1=sb_1p5[:],
                                     op=TT.add)
                    ep.tensor_tensor(out=y[:], in0=t1[:], in1=y[:], op=TT.mult)
                rs = y
                cc = stpool.tile([128, TPG], f32, tag="cc")
                ep.tensor_tensor(out=cc[:], in0=mvg[:, :, 0], in1=rs[:],
                                 op=TT.mult)
                ep.tensor_tensor(out=cc[:], in0=cc[:], in1=sb_neg1[:],
                                 op=TT.mult)

                # ---- normalize: o = h*rs + cc, split ACT/DVE ----
                n_act = NORM_ACT_CYCLE[g % len(NORM_ACT_CYCLE)]
                og = opool.tile([128, TPG, D], out_dt, tag="og")
                for t in range(TPG):
                    h = hg[:, t, :]
                    o = og[:, t, :]
                    if t < n_act:
                        nc.scalar.activation(
                            o, h,
                            mybir.ActivationFunctionType.Identity,
                            bias=cc[:, t : t + 1], scale=rs[:, t : t + 1])
                    else:
                        nc.vector.tensor_scalar(
                            out=o, in0=h,
                            scalar1=rs[:, t : t + 1], scalar2=cc[:, t : t + 1],
                            op0=mybir.AluOpType.mult,
                            op1=mybir.AluOpType.add,
                        )
                    if apply_affine:
                        nc.vector.tensor_mul(o, o, sb_gmb[:])
                        nc.vector.tensor_add(o, o, sb_btb[:])

                # one batched store: DRAM row a0 + t*128 + p <- og[p, t, :]
                out_view = out[a0 : a0 + A, :].rearrange(
                    "(t p) d -> p t d", p=128)
                nc.sync.dma_start(out=out_view, in_=og[:])

    nc.compile()
    return nc


def _get_module(npc: int, apply_affine: bool, sim_safe_silu: bool = False):
    key = (npc, apply_affine, sim_safe_silu)
    if key not in _MODULE_CACHE:
        _MODULE_CACHE[key] = _build_module(npc, apply_affine, sim_safe_silu)
    return _MODULE_CACHE[key]


def _prep_inputs(z, x, emb, w1, b1, w2, b2, gamma, beta, npc, apply_affine):
    """Host-side folding/transposes; returns per-core in_maps."""
    import ml_dtypes

    st = ml_dtypes.bfloat16

    z = np.asarray(z)
    x = np.asarray(x, dtype=np.float32)
    n = z.shape[0]

    xT = np.empty((4, n), dtype=np.float32)
    xT[0:3] = x.T
    xT[3] = 1.0
    xT = xT.astype(st)
    zi = np.asarray(z).astype(np.int64)
    ohT = (zi[None, :] == np.arange(NT, dtype=np.int64)[:, None]).astype(st)
    w1a = np.concatenate([np.asarray(w1, np.float32),
                          np.asarray(b1, np.float32).reshape(1, D)], axis=0)
    w1c = w1a.astype(st)
    w2f = np.asarray(w2, np.float32)
    w2a = np.stack([w2f[0:128], w2f[128:256]], axis=1).astype(st)
    emba = (np.asarray(emb, np.float32)
            + np.asarray(b2, np.float32).reshape(1, D)).astype(st)

    common = {"w1c": w1c, "w2a": w2a, "emba": emba}
    if apply_affine:
        common["gmb"] = np.broadcast_to(
            np.asarray(gamma, np.float32).reshape(1, D), (128, D)).copy()
        common["btb"] = np.broadcast_to(
            np.asarray(beta, np.float32).reshape(1, D), (128, D)).copy()

    in_maps = []
    for c in range(NCORES):
        s = slice(c * npc, (c + 1) * npc)
        m = {"xT": np.ascontiguousarray(xT[:, s]),
             "ohT": np.ascontiguousarray(ohT[:, s]), **common}
        in_maps.append(m)
    return in_maps


def _run(in_maps, nc, trace=False):
    from concourse.bass_interp import get_hw_module
    from concourse.bass_utils import run_bass_kernel_spmd

    old_m = nc.m
    nc.m = get_hw_module(nc.m)
    try:
        res = run_bass_kernel_spmd(
            nc, in_maps, core_ids=list(range(NCORES)), trace=trace
        )
    finally:
        nc.m = old_m
    return res


def kernel(z, x, emb, w1, b1, w2, b2, gamma, beta):
    z = np.asarray(z)
    x = np.asarray(x)
    assert z.shape[0] == N and x.shape == (N, 3), (z.shape, x.shape)

    apply_affine = not (
        np.all(np.asarray(gamma) == 1.0) and np.all(np.asarray(beta) == 0.0)
    )
    nc = _get_module(NPC, apply_affine)
    in_maps = _prep_inputs(z, x, emb, w1, b1, w2, b2, gamma, beta,
                           NPC, apply_affine)
    res = _run(in_maps, nc, trace=False)
    out = np.concatenate([np.asarray(r["out"]) for r in res.results], axis=0)
    return out.astype(np.float32)


# revision 12
# speedup vs baseline: 2.7778x; 1.1132x over previous
"""Trainium2 Bass kernel for AtomEmbedding:
    h = LayerNorm(emb[z] + W2 @ silu(W1 @ x + b1) + b2) * gamma + beta

v2 design (engine-balanced, bf16 output):
  - N = 524288 atoms sharded 65536/core over 8 NeuronCores; params replicated.
  - Groups of A=512 atoms; 2 groups in flight (PSUM: 2x pT(2 banks) +
    2x hg(2 banks) = 8 banks exactly).
  - mm1 computes p^T = [w1;b1]^T @ [x;1]^T into ONE PSUM tile [128,2,512];
    silu is a single ACT op over FD=1024 -> s bf16 SBUF.
  - mm2 (2 k-chunks) + one-hot embedding gather accumulate h tiles
    [128 atoms, 256] packed into one PSUM tile [128,4,256] (b2 folded into
    emb host-side; one-hot^T streamed bf16 from DRAM).
  - LayerNorm stats: 2x batched bn_stats ([128,2,256] -> [128,2,6], the
    512-elem HW cap) + 4x bn_aggr -> mean/var. rsqrt(var+eps) via
    0x5f3759df-seed Newton + cc = -mu*rs on the (otherwise idle) GpSimd
    engine, freeing the DVE.
  - Normalize o = h*rs + cc: tiles split between ACT (Identity, scale/bias
    per-partition APs) and DVE (tensor_scalar), alternating 3/1 and 2/2 per
    group to balance ACT (silu-loaded) vs DVE (stats-loaded).
  - Output written bf16 (halves store traffic; rel-err budget allows);
    host casts back to f32.
"""

import os
import sys

import numpy as np

for _p in ("/opt/trn_rl_repo", "/opt/pypackages"):
    if _p not in sys.path and os.path.isdir(_p):
        sys.path.append(_p)

N = 524288
D = 256
NT = 100  # number of atom types
NCORES = 8
NPC = N // NCORES  # atoms per core
A = int(os.environ.get("ATOMEMB_A", "512"))  # atoms per group
TPG = A // 128  # 128-atom tiles per group
EPS = 1e-5

# knobs (defaults = the design; env lets HW A/B without editing)
OUT_BF16 = os.environ.get("ATOMEMB_OUT_BF16", "1") == "1"
EPI_ENGINE = os.environ.get("ATOMEMB_EPI", "gpsimd")  # gpsimd | dve
NEWTON_ITERS = int(os.environ.get("ATOMEMB_NEWTON", "1"))
# normalize tiles on ACT per group, cycled (rest on DVE)
NORM_ACT_CYCLE = tuple(
    int(x) for x in os.environ.get("ATOMEMB_NORM_ACT", "3").split(","))
BUFS_IN = int(os.environ.get("ATOMEMB_BIN", "4"))
BUFS_OH = int(os.environ.get("ATOMEMB_BOH", "4"))
BUFS_S = int(os.environ.get("ATOMEMB_BS", "3"))
BUFS_O = int(os.environ.get("ATOMEMB_BO", "4"))
PSA_BUFS = int(os.environ.get("ATOMEMB_PSA", "1"))
PSB_BUFS = int(os.environ.get("ATOMEMB_PSB", "3"))
# engine issuing the input loads: sync (SP ring) frees the ACT sequencer
LOAD_ENG = os.environ.get("ATOMEMB_LOAD_ENG", "sync")

_MODULE_CACHE: dict = {}


def _build_module(npc: int, apply_affine: bool, sim_safe_silu: bool = False):
    """Build + compile the Bass module for one core's slice (npc atoms).

    sim_safe_silu: CoreSim doesn't implement the Silu activation; when True,
    emit Sigmoid + multiply instead (slower, only used for simulation runs).
    """
    from contextlib import ExitStack

    import concourse.bacc as bacc
    import concourse.tile as tile
    from concourse import mybir

    f32 = mybir.dt.float32
    bf16 = mybir.dt.bfloat16
    i32 = mybir.dt.int32
    out_dt = bf16 if OUT_BF16 else f32

    ngroups = npc // A
    assert npc % A == 0

    nc = bacc.Bacc(
        "TRN2",
        target_bir_lowering=False,
        debug=False,
        enable_asserts=False,
        num_devices=NCORES,
    )

    # Per-core inputs (host pre-transposed / folded):
    #   xT:   [4, npc]  rows = (x0, x1, x2, 1)     -> moving operand of mm1
    #   ohT:  [NT, npc] one-hot^T                   -> stationary of gather-mm
    #   w1c:  [4, D]    [w1; b1]                    -> stationary of mm1
    #   w2a:  [128, 2, D] w2 split into two k-chunks -> moving of mm2
    #   emba: [NT, D]   emb + b2                    -> moving of gather-mm
    xT = nc.dram_tensor("xT", [4, npc], bf16, kind="ExternalInput")
    ohT = nc.dram_tensor("ohT", [NT, npc], bf16, kind="ExternalInput")
    w1c = nc.dram_tensor("w1c", [4, D], bf16, kind="ExternalInput")
    w2a = nc.dram_tensor("w2a", [128, 2, D], bf16, kind="ExternalInput")
    emba = nc.dram_tensor("emba", [NT, D], bf16, kind="ExternalInput")
    if apply_affine:
        gmb = nc.dram_tensor("gmb", [128, D], f32, kind="ExternalInput")
        btb = nc.dram_tensor("btb", [128, D], f32, kind="ExternalInput")
    out = nc.dram_tensor("out", [npc, D], out_dt, kind="ExternalOutput")

    with tile.TileContext(nc) as tc:
        with ExitStack() as ctx:
            consts = ctx.enter_context(tc.tile_pool(name="consts", bufs=1))
            xpool = ctx.enter_context(tc.tile_pool(name="xpool", bufs=BUFS_IN))
            ohpool = ctx.enter_context(tc.tile_pool(name="ohpool", bufs=BUFS_OH))
            spool = ctx.enter_context(tc.tile_pool(name="spool", bufs=BUFS_S))
            stpool = ctx.enter_context(tc.tile_pool(name="stpool", bufs=4))
            opool = ctx.enter_context(tc.tile_pool(name="opool", bufs=BUFS_O))
            psA = ctx.enter_context(
                tc.tile_pool(name="psA", bufs=PSA_BUFS, space="PSUM"))
            psB = ctx.enter_context(
                tc.tile_pool(name="psB", bufs=PSB_BUFS, space="PSUM"))

            # ---- load constants once ----
            sb_w1 = consts.tile([4, D], bf16)
            nc.sync.dma_start(out=sb_w1[:], in_=w1c[:])
            sb_w2 = consts.tile([128, 2, D], bf16)
            nc.sync.dma_start(out=sb_w2[:], in_=w2a[:])
            sb_emb = consts.tile([NT, D], bf16)
            nc.sync.dma_start(out=sb_emb[:], in_=emba[:])
            sb_magic_i = consts.tile([128, TPG], i32)
            nc.vector.memset(sb_magic_i[:], 0x5F3759DF)
            sb_one_i = consts.tile([128, TPG], i32)
            nc.vector.memset(sb_one_i[:], 1)
            sb_eps = consts.tile([128, TPG], f32)
            nc.vector.memset(sb_eps[:], EPS)
            sb_nhalf = consts.tile([128, TPG], f32)
            nc.vector.memset(sb_nhalf[:], -0.5)
            sb_1p5 = consts.tile([128, TPG], f32)
            nc.vector.memset(sb_1p5[:], 1.5)
            sb_neg1 = consts.tile([128, TPG], f32)
            nc.vector.memset(sb_neg1[:], -1.0)
            if apply_affine:
                sb_gmb = consts.tile([128, D], f32)
                nc.sync.dma_start(out=sb_gmb[:], in_=gmb[:])
                sb_btb = consts.tile([128, D], f32)
                nc.sync.dma_start(out=sb_btb[:], in_=btb[:])

            # epilogue engine: ops on tiny [128, TPG] tiles
            ep = nc.gpsimd if EPI_ENGINE == "gpsimd" else nc.vector

            # Software-pipelined emission (engines execute their streams
            # in order, so emission order IS the schedule):
            #   iter it:  load(it)  |  mm1+silu(it-1)  |  mm2..store(it-2)
            # so when the PE reaches mm2(g), silu(g) completed a full group
            # period earlier -> the PE never waits on ACT and streams
            # continuously (keeps the p-state ramped).
            live: dict = {}

            def stage_load(g):
                a0 = g * A
                load_eng = nc.sync if LOAD_ENG == "sync" else nc.scalar
                xt = xpool.tile([4, A], bf16, tag="xt")
                load_eng.dma_start(out=xt[:], in_=xT[:, a0 : a0 + A])
                oh = ohpool.tile([NT, A], bf16, tag="oh")
                load_eng.dma_start(out=oh[:], in_=ohT[:, a0 : a0 + A])
                live[g] = {"xt": xt, "oh": oh}

            def stage_front(g):
                xt = live[g]["xt"]
                # mm1: p^T [128, 2, A] (D on partitions, one tile)
                pT = psA.tile([128, 2, A], f32, tag="pT")
                nc.tensor.matmul(pT[:, 0, :], sb_w1[:, 0:128], xt[:],
                                 start=True, stop=True)
                nc.tensor.matmul(pT[:, 1, :], sb_w1[:, 128:256], xt[:],
                                 start=True, stop=True)
                # silu on ACT (single op over FD=2A)
                s = spool.tile([128, 2, A], bf16, tag="s")
                if sim_safe_silu:
                    sg = spool.tile([128, 2, A], f32, tag="sg")
                    nc.scalar.activation(sg[:], pT[:],
                                         mybir.ActivationFunctionType.Sigmoid)
                    nc.vector.tensor_mul(s[:], sg[:], pT[:])
                else:
                    nc.scalar.activation(s[:], pT[:],
                                         mybir.ActivationFunctionType.Silu)
                live[g]["s"] = s

            def stage_back(g):
                a0 = g * A
                s, oh = live[g]["s"], live[g]["oh"]

                # mm2 + embedding gather: h tiles in one PSUM tile
                hg = psB.tile([128, TPG, D], f32, tag="hg")
                for t in range(TPG):
                    c = t * 128
                    nc.tensor.matmul(hg[:, t, :], s[:, 0, c : c + 128],
                                     sb_w2[:, 0, :], start=True, stop=False)
                    nc.tensor.matmul(hg[:, t, :], s[:, 1, c : c + 128],
                                     sb_w2[:, 1, :], start=False, stop=False)
                    nc.tensor.matmul(hg[:, t, :], oh[:, c : c + 128],
                                     sb_emb[:], start=False, stop=True)

                # ---- LayerNorm stats on DVE (bn_stats is single-group) ----
                stg = stpool.tile([128, TPG, 6], f32, tag="stg")
                for t in range(TPG):
                    nc.vector.bn_stats(out=stg[:, t, :], in_=hg[:, t, :])
                mvg = stpool.tile([128, TPG, 2], f32, tag="mvg")
                for t in range(TPG):
                    nc.vector.bn_aggr(out=mvg[:, t, :], in_=stg[:, t, :])

                # ---- epilogue: rs = rsqrt(var+eps), cc = -mu*rs ----
                # (bit-trick seed + Newton; mostly on GpSimd to free the DVE.
                #  Pool only supports plain tensor_tensor -> const-tile form;
                #  the int32 shift seed is DVE-only on TRN2.)
                TT = mybir.AluOpType
                w = stpool.tile([128, TPG], f32, tag="w")
                ep.tensor_tensor(out=w[:], in0=mvg[:, :, 1],
                                 in1=sb_eps[:], op=TT.add)
                y = stpool.tile([128, TPG], f32, tag="y")
                t1 = stpool.tile([128, TPG], f32, tag="t1")
                # yi = magic - (w_i32 >> 1)
                nc.vector.tensor_tensor(
                    out=t1[:].bitcast(i32), in0=w[:].bitcast(i32),
                    in1=sb_one_i[:, 0:TPG],
                    op=TT.logical_shift_right)
                nc.vector.tensor_tensor(
                    out=y[:].bitcast(i32), in0=sb_magic_i[:, 0:TPG],
                    in1=t1[:].bitcast(i32),
                    op=TT.subtract)
                # Newton: y <- y * (1.5 - 0.5*w*y*y)
                for _ in range(NEWTON_ITERS):
                    ep.tensor_tensor(out=t1[:], in0=y[:], in1=y[:], op=TT.mult)
                    ep.tensor_tensor(out=t1[:], in0=t1[:], in1=w[:], op=TT.mult)
                    ep.tensor_tensor(out=t1[:], in0=t1[:], in1=sb_nhalf[:],
                                     op=TT.mult)
                    ep.tensor_tensor(out=t1[:], in0=t1[:], in1=sb_1p5[:],
                                     op=TT.add)
                    ep.tensor_tensor(out=y[:], in0=t1[:], in1=y[:], op=TT.mult)
                rs = y
                cc = stpool.tile([128, TPG], f32, tag="cc")
                ep.tensor_tensor(out=cc[:], in0=mvg[:, :, 0], in1=rs[:],
                                 op=TT.mult)
                ep.tensor_tensor(out=cc[:], in0=cc[:], in1=sb_neg1[:],
                                 op=TT.mult)

                # ---- normalize: o = h*rs + cc, split ACT/DVE ----
                n_act = NORM_ACT_CYCLE[g % len(NORM_ACT_CYCLE)]
                og = opool.tile([128, TPG, D], out_dt, tag="og")
                for t in range(TPG):
                    h = hg[:, t, :]
                    o = og[:, t, :]
                    if t < n_act:
                        nc.scalar.activation(
                            o, h,
                            mybir.ActivationFunctionType.Identity,
                            bias=cc[:, t : t + 1], scale=rs[:, t : t + 1])
                    else:
                        nc.vector.tensor_scalar(
                            out=o, in0=h,
                            scalar1=rs[:, t : t + 1], scalar2=cc[:, t : t + 1],
                            op0=mybir.AluOpType.mult,
                            op1=mybir.AluOpType.add,
                        )
                    if apply_affine:
                        nc.vector.tensor_mul(o, o, sb_gmb[:])
                        nc.vector.tensor_add(o, o, sb_btb[:])

                # one batched store: DRAM row a0 + t*128 + p <- og[p, t, :]
                out_view = out[a0 : a0 + A, :].rearrange(
                    "(t p) d -> p t d", p=128)
                nc.sync.dma_start(out=out_view, in_=og[:])
                del live[g]

            for it in range(ngroups + 2):
                if it < ngroups:
                    stage_load(it)
                if 0 <= it - 1 < ngroups:
                    stage_front(it - 1)
                if 0 <= it - 2 < ngroups:
                    stage_back(it - 2)

    nc.compile()
    return nc


def _get_module(npc: int, apply_affine: bool, sim_safe_silu: bool = False):
    key = (npc, apply_affine, sim_safe_silu)
    if key not in _MODULE_CACHE:
        _MODULE_CACHE[key] = _build_module(npc, apply_affine, sim_safe_silu)
    return _MODULE_CACHE[key]


def _prep_inputs(z, x, emb, w1, b1, w2, b2, gamma, beta, npc, apply_affine):
    """Host-side folding/transposes; returns per-core in_maps."""
    import ml_dtypes

    st = ml_dtypes.bfloat16

    z = np.asarray(z)
    x = np.asarray(x, dtype=np.float32)
    n = z.shape[0]

    xT = np.empty((4, n), dtype=np.float32)
    xT[0:3] = x.T
    xT[3] = 1.0
    xT = xT.astype(st)
    zi = np.asarray(z).astype(np.int64)
    ohT = (zi[None, :] == np.arange(NT, dtype=np.int64)[:, None]).astype(st)
    w1a = np.concatenate([np.asarray(w1, np.float32),
                          np.asarray(b1, np.float32).reshape(1, D)], axis=0)
    w1c = w1a.astype(st)
    w2f = np.asarray(w2, np.float32)
    w2a = np.stack([w2f[0:128], w2f[128:256]], axis=1).astype(st)
    emba = (np.asarray(emb, np.float32)
            + np.asarray(b2, np.float32).reshape(1, D)).astype(st)

    common = {"w1c": w1c, "w2a": w2a, "emba": emba}
    if apply_affine:
        common["gmb"] = np.broadcast_to(
            np.asarray(gamma, np.float32).reshape(1, D), (128, D)).copy()
        common["btb"] = np.broadcast_to(
            np.asarray(beta, np.float32).reshape(1, D), (128, D)).copy()

    in_maps = []
    for c in range(NCORES):
        s = slice(c * npc, (c + 1) * npc)
        m = {"xT": np.ascontiguousarray(xT[:, s]),
             "ohT": np.ascontiguousarray(ohT[:, s]), **common}
        in_maps.append(m)
    return in_maps


def _run(in_maps, nc, trace=False):
    from concourse.bass_interp import get_hw_module
    from concourse.bass_utils import run_bass_kernel_spmd

    old_m = nc.m
    nc.m = get_hw_module(nc.m)
    try:
        res = run_bass_kernel_spmd(
            nc, in_maps, core_ids=list(range(NCORES)), trace=trace
        )
    finally:
        nc.m = old_m
    return res


def kernel(z, x, emb, w1, b1, w2, b2, gamma, beta):
    z = np.asarray(z)
    x = np.asarray(x)
    assert z.shape[0] == N and x.shape == (N, 3), (z.shape, x.shape)

    apply_affine = not (
        np.all(np.asarray(gamma) == 1.0) and np.all(np.asarray(beta) == 0.0)
    )
    nc = _get_module(NPC, apply_affine)
    in_maps = _prep_inputs(z, x, emb, w1, b1, w2, b2, gamma, beta,
                           NPC, apply_affine)
    res = _run(in_maps, nc, trace=False)
    out = np.concatenate([np.asarray(r["out"]) for r in res.results], axis=0)
    return out.astype(np.float32)


# revision 19
# speedup vs baseline: 3.3328x; 1.1998x over previous
"""Trainium2 Bass kernel for AtomEmbedding:
    h = LayerNorm(emb[z] + W2 @ silu(W1 @ x + b1) + b2) * gamma + beta

v2 design (engine-balanced, bf16 output):
  - N = 524288 atoms sharded 65536/core over 8 NeuronCores; params replicated.
  - Groups of A=512 atoms; 2 groups in flight (PSUM: 2x pT(2 banks) +
    2x hg(2 banks) = 8 banks exactly).
  - mm1 computes p^T = [w1;b1]^T @ [x;1]^T into ONE PSUM tile [128,2,512];
    silu is a single ACT op over FD=1024 -> s bf16 SBUF.
  - mm2 (2 k-chunks) + one-hot embedding gather accumulate h tiles
    [128 atoms, 256] packed into one PSUM tile [128,4,256] (b2 folded into
    emb host-side; one-hot^T streamed bf16 from DRAM).
  - LayerNorm stats: 2x batched bn_stats ([128,2,256] -> [128,2,6], the
    512-elem HW cap) + 4x bn_aggr -> mean/var. rsqrt(var+eps) via
    0x5f3759df-seed Newton + cc = -mu*rs on the (otherwise idle) GpSimd
    engine, freeing the DVE.
  - Normalize o = h*rs + cc: tiles split between ACT (Identity, scale/bias
    per-partition APs) and DVE (tensor_scalar), alternating 3/1 and 2/2 per
    group to balance ACT (silu-loaded) vs DVE (stats-loaded).
  - Output written bf16 (halves store traffic; rel-err budget allows);
    host casts back to f32.
"""

import os
import sys

import numpy as np

for _p in ("/opt/trn_rl_repo", "/opt/pypackages"):
    if _p not in sys.path and os.path.isdir(_p):
        sys.path.append(_p)

N = 524288
D = 256
NT = 100  # number of atom types
NCORES = 8
NPC = N // NCORES  # atoms per core
A = int(os.environ.get("ATOMEMB_A", "512"))  # atoms per group
TPG = A // 128  # 128-atom tiles per group
EPS = 1e-5

# knobs (defaults = the design; env lets HW A/B without editing)
OUT_BF16 = os.environ.get("ATOMEMB_OUT_BF16", "1") == "1"
EPI_ENGINE = os.environ.get("ATOMEMB_EPI", "gpsimd")  # gpsimd | dve
NEWTON_ITERS = int(os.environ.get("ATOMEMB_NEWTON", "1"))
# normalize tiles on ACT per group, cycled (rest on DVE)
NORM_ACT_CYCLE = tuple(
    int(x) for x in os.environ.get("ATOMEMB_NORM_ACT", "3").split(","))
BUFS_IN = int(os.environ.get("ATOMEMB_BIN", "4"))
BUFS_OH = int(os.environ.get("ATOMEMB_BOH", "4"))
BUFS_S = int(os.environ.get("ATOMEMB_BS", "3"))
BUFS_O = int(os.environ.get("ATOMEMB_BO", "4"))
PSA_BUFS = int(os.environ.get("ATOMEMB_PSA", "2"))
PSB_BUFS = int(os.environ.get("ATOMEMB_PSB", "2"))
# engine issuing the input loads: sync (SP ring) frees the ACT sequencer
LOAD_ENG = os.environ.get("ATOMEMB_LOAD_ENG", "sync")

_MODULE_CACHE: dict = {}


def _build_module(npc: int, apply_affine: bool, sim_safe_silu: bool = False):
    """Build + compile the Bass module for one core's slice (npc atoms).

    sim_safe_silu: CoreSim doesn't implement the Silu activation; when True,
    emit Sigmoid + multiply instead (slower, only used for simulation runs).
    """
    from contextlib import ExitStack

    import concourse.bacc as bacc
    import concourse.tile as tile
    from concourse import mybir

    f32 = mybir.dt.float32
    bf16 = mybir.dt.bfloat16
    i32 = mybir.dt.int32
    out_dt = bf16 if OUT_BF16 else f32

    ngroups = npc // A
    assert npc % A == 0

    nc = bacc.Bacc(
        "TRN2",
        target_bir_lowering=False,
        debug=False,
        enable_asserts=False,
        num_devices=NCORES,
    )

    # Per-core inputs (host pre-transposed / folded):
    #   xT:   [4, npc]  rows = (x0, x1, x2, 1)     -> moving operand of mm1
    #   ohT:  [NT, npc] one-hot^T                   -> stationary of gather-mm
    #   w1c:  [4, D]    [w1; b1]                    -> stationary of mm1
    #   w2a:  [128, 2, D] w2 split into two k-chunks -> moving of mm2
    #   emba: [NT, D]   emb + b2                    -> moving of gather-mm
    xT = nc.dram_tensor("xT", [4, npc], bf16, kind="ExternalInput")
    ohT = nc.dram_tensor("ohT", [NT, npc], bf16, kind="ExternalInput")
    w1c = nc.dram_tensor("w1c", [4, D], bf16, kind="ExternalInput")
    w2a = nc.dram_tensor("w2a", [128, 2, D], bf16, kind="ExternalInput")
    emba = nc.dram_tensor("emba", [NT, D], bf16, kind="ExternalInput")
    if apply_affine:
        gmb = nc.dram_tensor("gmb", [128, D], f32, kind="ExternalInput")
        btb = nc.dram_tensor("btb", [128, D], f32, kind="ExternalInput")
    out = nc.dram_tensor("out", [npc, D], out_dt, kind="ExternalOutput")

    with tile.TileContext(nc) as tc:
        with ExitStack() as ctx:
            consts = ctx.enter_context(tc.tile_pool(name="consts", bufs=1))
            xpool = ctx.enter_context(tc.tile_pool(name="xpool", bufs=BUFS_IN))
            ohpool = ctx.enter_context(tc.tile_pool(name="ohpool", bufs=BUFS_OH))
            spool = ctx.enter_context(tc.tile_pool(name="spool", bufs=BUFS_S))
            stpool = ctx.enter_context(tc.tile_pool(name="stpool", bufs=4))
            opool = ctx.enter_context(tc.tile_pool(name="opool", bufs=BUFS_O))
            psA = ctx.enter_context(
                tc.tile_pool(name="psA", bufs=PSA_BUFS, space="PSUM"))
            psB = ctx.enter_context(
                tc.tile_pool(name="psB", bufs=PSB_BUFS, space="PSUM"))

            # ---- load constants once ----
            sb_w1 = consts.tile([4, D], bf16)
            nc.sync.dma_start(out=sb_w1[:], in_=w1c[:])
            sb_w2 = consts.tile([128, 2, D], bf16)
            nc.sync.dma_start(out=sb_w2[:], in_=w2a[:])
            sb_emb = consts.tile([NT, D], bf16)
            nc.sync.dma_start(out=sb_emb[:], in_=emba[:])
            sb_magic_i = consts.tile([128, TPG], i32)
            nc.vector.memset(sb_magic_i[:], 0x5F3759DF)
            sb_one_i = consts.tile([128, TPG], i32)
            nc.vector.memset(sb_one_i[:], 1)
            sb_eps = consts.tile([128, TPG], f32)
            nc.vector.memset(sb_eps[:], EPS)
            sb_nhalf = consts.tile([128, TPG], f32)
            nc.vector.memset(sb_nhalf[:], -0.5)
            sb_1p5 = consts.tile([128, TPG], f32)
            nc.vector.memset(sb_1p5[:], 1.5)
            sb_neg1 = consts.tile([128, TPG], f32)
            nc.vector.memset(sb_neg1[:], -1.0)
            if apply_affine:
                sb_gmb = consts.tile([128, D], f32)
                nc.sync.dma_start(out=sb_gmb[:], in_=gmb[:])
                sb_btb = consts.tile([128, D], f32)
                nc.sync.dma_start(out=sb_btb[:], in_=btb[:])

            # epilogue engine: ops on tiny [128, TPG] tiles
            ep = nc.gpsimd if EPI_ENGINE == "gpsimd" else nc.vector

            # Software-pipelined emission (engines execute their streams
            # in order, so emission order IS the schedule):
            #   iter it:  load(it)  |  mm1+silu(it-1)  |  mm2..store(it-2)
            # so when the PE reaches mm2(g), silu(g) completed a full group
            # period earlier -> the PE never waits on ACT and streams
            # continuously (keeps the p-state ramped).
            live: dict = {}

            def stage_load(g):
                a0 = g * A
                load_eng = nc.sync if LOAD_ENG == "sync" else nc.scalar
                xt = xpool.tile([4, A], bf16, tag="xt")
                load_eng.dma_start(out=xt[:], in_=xT[:, a0 : a0 + A])
                oh = ohpool.tile([NT, A], bf16, tag="oh")
                load_eng.dma_start(out=oh[:], in_=ohT[:, a0 : a0 + A])
                live[g] = {"xt": xt, "oh": oh}

            def stage_front(g):
                xt = live[g]["xt"]
                # mm1: p^T [128, 2, A] (D on partitions, one tile)
                pT = psA.tile([128, 2, A], f32, tag="pT")
                nc.tensor.matmul(pT[:, 0, :], sb_w1[:, 0:128], xt[:],
                                 start=True, stop=True)
                nc.tensor.matmul(pT[:, 1, :], sb_w1[:, 128:256], xt[:],
                                 start=True, stop=True)
                # silu on ACT (single op over FD=2A)
                s = spool.tile([128, 2, A], bf16, tag="s")
                if sim_safe_silu:
                    sg = spool.tile([128, 2, A], f32, tag="sg")
                    nc.scalar.activation(sg[:], pT[:],
                                         mybir.ActivationFunctionType.Sigmoid)
                    nc.vector.tensor_mul(s[:], sg[:], pT[:])
                else:
                    nc.scalar.activation(s[:], pT[:],
                                         mybir.ActivationFunctionType.Silu)
                live[g]["s"] = s

            def stage_back(g):
                a0 = g * A
                s, oh = live[g]["s"], live[g]["oh"]

                # mm2 + embedding gather: h as 2 one-bank PSUM tiles (pairs
                # of 128-atom tiles) so stats can start after 6 matmuls
                # (PSUM pools are bank-granular; half-bank tiles would
                # waste a bank each)
                hp = [psB.tile([128, 2, D], f32, tag=f"hp{p}", name=f"hp{p}")
                      for p in range(TPG // 2)]
                hs = [hp[t // 2][:, t % 2, :] for t in range(TPG)]
                for t in range(TPG):
                    c = t * 128
                    h = hs[t]
                    nc.tensor.matmul(h, s[:, 0, c : c + 128],
                                     sb_w2[:, 0, :], start=True, stop=False)
                    nc.tensor.matmul(h, s[:, 1, c : c + 128],
                                     sb_w2[:, 1, :], start=False, stop=False)
                    nc.tensor.matmul(h, oh[:, c : c + 128],
                                     sb_emb[:], start=False, stop=True)

                # ---- LayerNorm stats on DVE (bn_stats is single-group) ----
                stg = stpool.tile([128, TPG, 6], f32, tag="stg")
                for t in range(TPG):
                    nc.vector.bn_stats(out=stg[:, t, :], in_=hs[t])
                mvg = stpool.tile([128, TPG, 2], f32, tag="mvg")
                for t in range(TPG):
                    nc.vector.bn_aggr(out=mvg[:, t, :], in_=stg[:, t, :])

                # ---- epilogue: rs = rsqrt(var+eps), cc = -mu*rs ----
                # (bit-trick seed + Newton; mostly on GpSimd to free the DVE.
                #  Pool only supports plain tensor_tensor -> const-tile form;
                #  the int32 shift seed is DVE-only on TRN2.)
                TT = mybir.AluOpType
                w = stpool.tile([128, TPG], f32, tag="w")
                ep.tensor_tensor(out=w[:], in0=mvg[:, :, 1],
                                 in1=sb_eps[:], op=TT.add)
                y = stpool.tile([128, TPG], f32, tag="y")
                t1 = stpool.tile([128, TPG], f32, tag="t1")
                # yi = magic - (w_i32 >> 1)
                nc.vector.tensor_tensor(
                    out=t1[:].bitcast(i32), in0=w[:].bitcast(i32),
                    in1=sb_one_i[:, 0:TPG],
                    op=TT.logical_shift_right)
                nc.vector.tensor_tensor(
                    out=y[:].bitcast(i32), in0=sb_magic_i[:, 0:TPG],
                    in1=t1[:].bitcast(i32),
                    op=TT.subtract)
                # Newton: y <- y * (1.5 - 0.5*w*y*y)
                for _ in range(NEWTON_ITERS):
                    ep.tensor_tensor(out=t1[:], in0=y[:], in1=y[:], op=TT.mult)
                    ep.tensor_tensor(out=t1[:], in0=t1[:], in1=w[:], op=TT.mult)
                    ep.tensor_tensor(out=t1[:], in0=t1[:], in1=sb_nhalf[:],
                                     op=TT.mult)
                    ep.tensor_tensor(out=t1[:], in0=t1[:], in1=sb_1p5[:],
                                     op=TT.add)
                    ep.tensor_tensor(out=y[:], in0=t1[:], in1=y[:], op=TT.mult)
                rs = y
                cc = stpool.tile([128, TPG], f32, tag="cc")
                ep.tensor_tensor(out=cc[:], in0=mvg[:, :, 0], in1=rs[:],
                                 op=TT.mult)
                ep.tensor_tensor(out=cc[:], in0=cc[:], in1=sb_neg1[:],
                                 op=TT.mult)

                # ---- normalize: o = h*rs + cc, split ACT/DVE ----
                n_act = NORM_ACT_CYCLE[g % len(NORM_ACT_CYCLE)]
                og = opool.tile([128, TPG, D], out_dt, tag="og")
                for t in range(TPG):
                    h = hs[t]
                    o = og[:, t, :]
                    if t < n_act:
                        nc.scalar.activation(
                            o, h,
                            mybir.ActivationFunctionType.Identity,
                            bias=cc[:, t : t + 1], scale=rs[:, t : t + 1])
                    else:
                        nc.vector.tensor_scalar(
                            out=o, in0=h,
                            scalar1=rs[:, t : t + 1], scalar2=cc[:, t : t + 1],
                            op0=mybir.AluOpType.mult,
                            op1=mybir.AluOpType.add,
                        )
                    if apply_affine:
                        nc.vector.tensor_mul(o, o, sb_gmb[:])
                        nc.vector.tensor_add(o, o, sb_btb[:])

                # one batched store: DRAM row a0 + t*128 + p <- og[p, t, :]
                out_view = out[a0 : a0 + A, :].rearrange(
                    "(t p) d -> p t d", p=128)
                nc.sync.dma_start(out=out_view, in_=og[:])
                del live[g]

            for it in range(ngroups + 2):
                if it < ngroups:
                    stage_load(it)
                if 0 <= it - 1 < ngroups:
                    stage_front(it - 1)
                if 0 <= it - 2 < ngroups:
                    stage_back(it - 2)

    nc.compile()
    return nc


def _get_module(npc: int, apply_affine: bool, sim_safe_silu: bool = False):
    key = (npc, apply_affine, sim_safe_silu)
    if key not in _MODULE_CACHE:
        _MODULE_CACHE[key] = _build_module(npc, apply_affine, sim_safe_silu)
    return _MODULE_CACHE[key]


def _prep_inputs(z, x, emb, w1, b1, w2, b2, gamma, beta, npc, apply_affine):
    """Host-side folding/transposes; returns per-core in_maps."""
    import ml_dtypes

    st = ml_dtypes.bfloat16

    z = np.asarray(z)
    x = np.asarray(x, dtype=np.float32)
    n = z.shape[0]

    xT = np.empty((4, n), dtype=np.float32)
    xT[0:3] = x.T
    xT[3] = 1.0
    xT = xT.astype(st)
    zi = np.asarray(z).astype(np.int64)
    ohT = (zi[None, :] == np.arange(NT, dtype=np.int64)[:, None]).astype(st)
    w1a = np.concatenate([np.asarray(w1, np.float32),
                          np.asarray(b1, np.float32).reshape(1, D)], axis=0)
    w1c = w1a.astype(st)
    w2f = np.asarray(w2, np.float32)
    w2a = np.stack([w2f[0:128], w2f[128:256]], axis=1).astype(st)
    emba = (np.asarray(emb, np.float32)
            + np.asarray(b2, np.float32).reshape(1, D)).astype(st)

    common = {"w1c": w1c, "w2a": w2a, "emba": emba}
    if apply_affine:
        common["gmb"] = np.broadcast_to(
            np.asarray(gamma, np.float32).reshape(1, D), (128, D)).copy()
        common["btb"] = np.broadcast_to(
            np.asarray(beta, np.float32).reshape(1, D), (128, D)).copy()

    in_maps = []
    for c in range(NCORES):
        s = slice(c * npc, (c + 1) * npc)
        m = {"xT": np.ascontiguousarray(xT[:, s]),
             "ohT": np.ascontiguousarray(ohT[:, s]), **common}
        in_maps.append(m)
    return in_maps


def _run(in_maps, nc, trace=False):
    from concourse.bass_interp import get_hw_module
    from concourse.bass_utils import run_bass_kernel_spmd

    old_m = nc.m
    nc.m = get_hw_module(nc.m)
    try:
        res = run_bass_kernel_spmd(
            nc, in_maps, core_ids=list(range(NCORES)), trace=trace
        )
    finally:
        nc.m = old_m
    return res


def kernel(z, x, emb, w1, b1, w2, b2, gamma, beta):
    z = np.asarray(z)
    x = np.asarray(x)
    assert z.shape[0] == N and x.shape == (N, 3), (z.shape, x.shape)

    apply_affine = not (
        np.all(np.asarray(gamma) == 1.0) and np.all(np.asarray(beta) == 0.0)
    )
    nc = _get_module(NPC, apply_affine)
    in_maps = _prep_inputs(z, x, emb, w1, b1, w2, b2, gamma, beta,
                           NPC, apply_affine)
    res = _run(in_maps, nc, trace=False)
    out = np.concatenate([np.asarray(r["out"]) for r in res.results], axis=0)
    return out.astype(np.float32)
